# revision 5
# baseline (speedup 1.0000x reference)
"""GRU cell kernel for Trainium2, 8-core data-parallel.

Layout: activations staged feature-major ([128, B]) by the host so the device
streams them straight into the tensor engine (contraction over the 128-feature
partition dim); per-partition biases are fused into the ScalarE activations.

Gate dtype plans (CONFIG["plan"]):
  "A": z,r gates via one fp8e4 DoubleRow matmul each (K=256 contraction of the
       interleaved [x8;h8] pair), h-candidate via fp8 Wh*x8 + bf16 Uh*rh.
       PE cost 4 matmul-columns per batch column (vs 6 at bf16).
  "B": like A but the z gate runs fp8 Wz*x8 + bf16 Uz*h (cleaner z numerics).
  "C": all-bf16 (baseline numerics), no fp8 staging.

HBM traffic per core: pair 4MiB (A/B) or x 4MiB (C), h 4MiB, out 4MiB.
"""

from contextlib import ExitStack

import numpy as np

B = 131072
H = 128
NCORES = 8
BC = B // NCORES  # 16384 batch columns per core
CHUNK = 512  # matmul free dim (one PSUM bank)

CONFIG = {
    "plan": "C",
    "free": 1024,  # ACT/DVE/DMA tile width
    "taper": 512,  # first/last chunk width (pipeline fill/drain)
    "io_bufs": 10,
    "mid_bufs": 6,
}

_CACHE = {}
LAST_RESULTS = None


def _build_program(n_passes=1, mode="full", cfg=None):
    import concourse.bass as bass
    import concourse.tile as tile
    from concourse import bacc, mybir

    cfg = dict(CONFIG, **(cfg or {}))
    plan = cfg["plan"]

    f32 = mybir.dt.float32
    bf16 = mybir.dt.bfloat16
    fp8 = mybir.dt.float8e4
    DR = mybir.MatmulPerfMode.DoubleRow

    Sig = mybir.ActivationFunctionType.Sigmoid
    Tanh = mybir.ActivationFunctionType.Tanh
    Mult = mybir.AluOpType.mult
    Sub = mybir.AluOpType.subtract
    Add = mybir.AluOpType.add

    nc = bacc.Bacc(
        "TRN2",
        target_bir_lowering=False,
        debug=False,
        enable_asserts=False,
        num_devices=NCORES,
    )

    # Weight order on the stacked dim: Wz, Uz, Wr, Ur, Wh, Uh.
    Wz_i, Uz_i, Wr_i, Ur_i, Wh_i, Uh_i = range(6)
    if plan in ("A", "B"):
        XH = nc.dram_tensor("XH", [H, 2, BC], fp8, kind="ExternalInput").ap()
        W8 = nc.dram_tensor("W8", [H, 6, H], fp8, kind="ExternalInput").ap()
    else:
        XT = nc.dram_tensor("xT", [H, BC], bf16, kind="ExternalInput").ap()
    WB = nc.dram_tensor("WB", [H, 6, H], bf16, kind="ExternalInput").ap()
    HT = nc.dram_tensor("hT", [H, BC], bf16, kind="ExternalInput").ap()
    BIAS = nc.dram_tensor("bias", [H, 3], f32, kind="ExternalInput").ap()
    OT = nc.dram_tensor("oT", [H, BC], bf16, kind="ExternalOutput").ap()

    FREE = cfg["free"]
    taper = cfg.get("taper") or FREE
    # Chunk list: tapered first/last chunks shorten pipeline fill/drain.
    widths = []
    if taper < FREE:
        widths.append(taper)
    body = (BC - 2 * taper) // FREE if taper < FREE else BC // FREE
    widths += [FREE] * body
    rem = BC - sum(widths) - (taper if taper < FREE else 0)
    assert rem == 0, (widths, rem)
    if taper < FREE:
        widths.append(taper)
    offs = np.cumsum([0] + widths[:-1]).tolist()
    chunks = list(zip(offs, widths))

    with tile.TileContext(nc) as tc:
        with ExitStack() as ctx:
            consts = ctx.enter_context(tc.tile_pool(name="consts", bufs=1))
            io = ctx.enter_context(tc.tile_pool(name="io", bufs=cfg["io_bufs"]))
            mid = ctx.enter_context(tc.tile_pool(name="mid", bufs=cfg["mid_bufs"]))
            psum = ctx.enter_context(tc.tile_pool(name="psum", bufs=2, space="PSUM"))

            wb = consts.tile([H, 6, H], bf16)
            nc.sync.dma_start(wb[:], WB)
            if plan in ("A", "B"):
                w8 = consts.tile([H, 6, H], fp8)
                nc.sync.dma_start(w8[:], W8)
            b_s = consts.tile([H, 3], f32)
            nc.sync.dma_start(b_s[:], BIAS)
            bz, br, bh = (b_s[:, i : i + 1] for i in range(3))

            carry = None

            def emit_tail(s):
                # Uh (bf16) accumulation closes the h-candidate PSUM, then
                # tanh + blend + store for the carried chunk.
                w = s["w"]
                for ss in s["mm"]:
                    nc.tensor.matmul(
                        s["ph"][:, ss], wb[:, Uh_i, :], s["rh"][:, ss],
                        start=False, stop=True,
                    )
                hc = mid.tile([H, FREE], bf16, tag="hc")
                nc.scalar.activation(hc[:, :w], s["ph"][:, :w], Tanh, bias=bh)
                d = mid.tile([H, FREE], bf16, tag="d")
                nc.vector.tensor_tensor(d[:, :w], hc[:, :w], s["ht"][:, :w], Sub)
                m = mid.tile([H, FREE], bf16, tag="m")
                nc.vector.tensor_tensor(m[:, :w], s["z"][:, :w], d[:, :w], Mult)
                o = mid.tile([H, FREE], bf16, tag="o")
                nc.vector.tensor_tensor(o[:, :w], s["ht"][:, :w], m[:, :w], Add)
                nc.sync.dma_start(OT[:, s["sl"]], o[:, :w])

            for off, w in chunks * n_passes:
                sl = slice(off, off + w)
                mm = [slice(s, s + min(CHUNK, w - s)) for s in range(0, w, CHUNK)]
                ht = io.tile([H, FREE], bf16, tag="ht")
                nc.sync.dma_start(ht[:, :w], HT[:, sl])
                if plan in ("A", "B"):
                    mt = io.tile([H, 2, FREE], fp8, tag="mt")
                    nc.sync.dma_start(mt[:, :, :w], XH[:, :, sl])
                    xs = mt[:, 0, :]
                else:
                    xt = io.tile([H, FREE], bf16, tag="xt")
                    nc.sync.dma_start(xt[:, :w], XT[:, sl])
                    xs = xt[:]

                if mode == "dma":
                    o = mid.tile([H, FREE], bf16, tag="o")
                    nc.vector.tensor_copy(o[:, :w], ht[:, :w])
                    nc.sync.dma_start(OT[:, sl], o[:, :w])
                    continue

                pz = psum.tile([H, FREE], f32, tag="pz", bufs=1)
                pr = psum.tile([H, FREE], f32, tag="pr", bufs=1)
                ph = psum.tile([H, FREE], f32, tag="ph", bufs=2)

                for ss in mm:
                    if plan == "A":
                        nc.tensor.matmul(pz[:, ss], w8[:, Wz_i : Wz_i + 2, :],
                                         mt[:, :, ss], start=True, stop=True,
                                         perf_mode=DR)
                    elif plan == "B":
                        nc.tensor.matmul(pz[:, ss], w8[:, Wz_i, :],
                                         mt[:, 0, ss], start=True, stop=False)
                        nc.tensor.matmul(pz[:, ss], wb[:, Uz_i, :],
                                         ht[:, ss], start=False, stop=True)
                    else:
                        nc.tensor.matmul(pz[:, ss], wb[:, Wz_i, :],
                                         xs[:, ss], start=True, stop=False)
                        nc.tensor.matmul(pz[:, ss], wb[:, Uz_i, :],
                                         ht[:, ss], start=False, stop=True)
                for ss in mm:
                    if plan in ("A", "B"):
                        nc.tensor.matmul(pr[:, ss], w8[:, Wr_i : Wr_i + 2, :],
                                         mt[:, :, ss], start=True, stop=True,
                                         perf_mode=DR)
                    else:
                        nc.tensor.matmul(pr[:, ss], wb[:, Wr_i, :],
                                         xs[:, ss], start=True, stop=False)
                        nc.tensor.matmul(pr[:, ss], wb[:, Ur_i, :],
                                         ht[:, ss], start=False, stop=True)
                for ss in mm:
                    if plan in ("A", "B"):
                        nc.tensor.matmul(ph[:, ss], w8[:, Wh_i, :],
                                         mt[:, 0, ss], start=True, stop=False)
                    else:
                        nc.tensor.matmul(ph[:, ss], wb[:, Wh_i, :],
                                         xs[:, ss], start=True, stop=False)

                z = mid.tile([H, FREE], bf16, tag="z")
                nc.scalar.activation(z[:, :w], pz[:, :w], Sig, bias=bz)
                r = mid.tile([H, FREE], bf16, tag="r")
                nc.scalar.activation(r[:, :w], pr[:, :w], Sig, bias=br)

                rh = mid.tile([H, FREE], bf16, tag="rh")
                nc.vector.tensor_tensor(rh[:, :w], r[:, :w], ht[:, :w], Mult)

                # Software-pipeline the Uh+tanh+blend stage by one chunk so
                # the in-order PE stream never waits on this chunk's rh.
                if carry is not None:
                    emit_tail(carry)
                carry = dict(ph=ph, rh=rh, z=z, ht=ht, sl=sl, w=w, mm=mm)
            if carry is not None:
                emit_tail(carry)

    nc.compile()
    return nc


def _get_program(n_passes=1, mode="full", cfg=None):
    key = (n_passes, mode, tuple(sorted((cfg or CONFIG).items())))
    if key not in _CACHE:
        _CACHE[key] = _build_program(n_passes, mode, cfg)
    return _CACHE[key]


def make_in_maps(x_t, h_prev, Wz, Uz, bz, Wr, Ur, br, Wh, Uh, bh, cfg=None):
    import ml_dtypes

    cfg = dict(CONFIG, **(cfg or {}))
    plan = cfg["plan"]
    bf = ml_dtypes.bfloat16
    f8 = ml_dtypes.float8_e4m3

    W = np.empty((H, 6, H), dtype=np.float32)
    for i, w in enumerate((Wz, Uz, Wr, Ur, Wh, Uh)):
        W[:, i, :] = np.asarray(w, dtype=np.float32)
    bias = np.empty((H, 3), dtype=np.float32)
    for i, b in enumerate((bz, br, bh)):
        bias[:, i] = np.asarray(b, dtype=np.float32)

    xT = np.asarray(x_t, dtype=np.float32).T
    hT = np.asarray(h_prev, dtype=np.float32).T
    hTb = np.ascontiguousarray(hT.astype(bf))
    if plan in ("A", "B"):
        xh = np.empty((H, 2, B), dtype=f8)
        xh[:, 0, :] = xT.astype(f8)
        xh[:, 1, :] = hT.astype(f8)
        W8 = W.astype(f8)
    else:
        xTb = np.ascontiguousarray(xT.astype(bf))
    WBv = W.astype(bf)

    in_maps = []
    for c in range(NCORES):
        sl = slice(c * BC, (c + 1) * BC)
        m = {
            "hT": np.ascontiguousarray(hTb[:, sl]),
            "WB": WBv,
            "bias": bias,
        }
        if plan in ("A", "B"):
            m["XH"] = np.ascontiguousarray(xh[:, :, sl])
            m["W8"] = W8
        else:
            m["xT"] = np.ascontiguousarray(xTb[:, sl])
        in_maps.append(m)
    return in_maps


def kernel(x_t, h_prev, Wz, Uz, bz, Wr, Ur, br, Wh, Uh, bh):
    global LAST_RESULTS
    from concourse import bass_utils

    in_maps = make_in_maps(x_t, h_prev, Wz, Uz, bz, Wr, Ur, br, Wh, Uh, bh)
    nc = _get_program()
    res = bass_utils.run_bass_kernel_spmd(nc, in_maps, core_ids=list(range(NCORES)))
    LAST_RESULTS = res

    oT = np.concatenate([r["oT"] for r in res.results], axis=1)  # [H, B]
    return np.ascontiguousarray(oT.T.astype(np.float32))


# revision 9
# speedup vs baseline: 1.0197x; 1.0197x over previous
"""GRU cell kernel for Trainium2, 8-core data-parallel.

All activations are staged feature-major ([128, B]) by the host so the device
streams them straight into the tensor engine (contraction over the 128-feature
partition dim). Per chunk, x and h arrive in ONE contiguous DMA (host
interleaves them per-chunk: [x_chunk | h_chunk]), biases ride in the weight
tensor, and gate biases are fused into the ScalarE activations.

Math (all bf16 operands, fp32 PSUM accumulate):
  z = sigmoid(Wz.T x + Uz.T h + bz)        2 matmuls -> PSUM, 1 ACT
  r = sigmoid(Wr.T x + Ur.T h + br)        2 matmuls, 1 ACT
  hc = tanh(Wh.T x + Uh.T (r*h) + bh)      2 matmuls, 1 ACT, 1 DVE mult
  h' = h + z*(hc - h)                      3 DVE ops
The Uh/tanh/blend stage is software-pipelined one chunk behind so the
in-order PE stream never waits on the DVE's r*h of the current chunk.

Engine budget per core (16384 batch cols): ACT ~48us (pacer), PE ~47us,
DVE ~43us, DMA ~36us. Head/tail chunks are tapered to shorten pipeline
fill/drain.
"""

from contextlib import ExitStack

import numpy as np

B = 131072
H = 128
NCORES = 8
BC = B // NCORES  # 16384 batch columns per core
CHUNK = 512  # matmul free dim (one PSUM bank)

CONFIG = {
    "free": 1024,  # ACT/DVE/DMA tile width
    "head": (512, 512),  # tapered leading chunk widths (pipeline fill)
    "tail": (512, 512),  # tapered trailing chunk widths (pipeline drain)
    "io_bufs": 10,
    "mid_bufs": 6,
}

_CACHE = {}
LAST_RESULTS = None


def _chunks(cfg):
    FREE = cfg["free"]
    head = list(cfg.get("head") or ())
    tail = list(cfg.get("tail") or ())
    body = (BC - sum(head) - sum(tail)) // FREE
    widths = head + [FREE] * body + tail
    assert sum(widths) == BC, widths
    offs = np.cumsum([0] + widths[:-1]).tolist()
    return list(zip(offs, widths))


def _build_program(n_passes=1, mode="full", cfg=None):
    import concourse.bass as bass  # noqa: F401
    import concourse.tile as tile
    from concourse import bacc, mybir

    cfg = dict(CONFIG, **(cfg or {}))

    f32 = mybir.dt.float32
    bf16 = mybir.dt.bfloat16

    Sig = mybir.ActivationFunctionType.Sigmoid
    Tanh = mybir.ActivationFunctionType.Tanh
    Mult = mybir.AluOpType.mult
    Sub = mybir.AluOpType.subtract
    Add = mybir.AluOpType.add

    nc = bacc.Bacc(
        "TRN2",
        target_bir_lowering=False,
        debug=False,
        enable_asserts=False,
        num_devices=NCORES,
    )

    # Weight order on the stacked dim: Wz, Uz, Wr, Ur, Wh, Uh; 3 bias columns
    # (bz, br, bh) appended.
    Wz_i, Uz_i, Wr_i, Ur_i, Wh_i, Uh_i = range(6)
    WBB = nc.dram_tensor("WBB", [H, 6 * H + 3], bf16, kind="ExternalInput").ap()
    XHB = nc.dram_tensor("XHB", [H, 2 * BC], bf16, kind="ExternalInput").ap()
    OT = nc.dram_tensor("oT", [H, BC], bf16, kind="ExternalOutput").ap()

    FREE = cfg["free"]
    chunks = _chunks(cfg)

    with tile.TileContext(nc) as tc:
        with ExitStack() as ctx:
            consts = ctx.enter_context(tc.tile_pool(name="consts", bufs=1))
            io = ctx.enter_context(tc.tile_pool(name="io", bufs=cfg["io_bufs"]))
            mid = ctx.enter_context(tc.tile_pool(name="mid", bufs=cfg["mid_bufs"]))
            psum = ctx.enter_context(tc.tile_pool(name="psum", bufs=2, space="PSUM"))

            wbb = consts.tile([H, 6 * H + 3], bf16)
            nc.sync.dma_start(wbb[:], WBB)
            wm = [wbb[:, i * H : (i + 1) * H] for i in range(6)]
            bz, br, bh = (wbb[:, 6 * H + i : 6 * H + i + 1] for i in range(3))

            carry = None

            def emit_tail(s):
                # Uh accumulation closes the h-candidate PSUM, then
                # tanh + blend + store for the carried chunk.
                w = s["w"]
                for ss in s["mm"]:
                    nc.tensor.matmul(
                        s["ph"][:, ss], wm[Uh_i], s["rh"][:, ss],
                        start=False, stop=True,
                    )
                hc = mid.tile([H, FREE], bf16, tag="hc")
                nc.scalar.activation(hc[:, :w], s["ph"][:, :w], Tanh, bias=bh)
                d = mid.tile([H, FREE], bf16, tag="d")
                nc.vector.tensor_tensor(d[:, :w], hc[:, :w], s["hs"], Sub)
                m = mid.tile([H, FREE], bf16, tag="m")
                nc.vector.tensor_tensor(m[:, :w], s["z"][:, :w], d[:, :w], Mult)
                o = mid.tile([H, FREE], bf16, tag="o")
                nc.vector.tensor_tensor(o[:, :w], s["hs"], m[:, :w], Add)
                nc.sync.dma_start(OT[:, s["sl"]], o[:, :w])

            for off, w in chunks * n_passes:
                sl = slice(off, off + w)
                mm = [slice(s, s + min(CHUNK, w - s)) for s in range(0, w, CHUNK)]
                xh = io.tile([H, 2 * FREE], bf16, tag="xh")
                nc.sync.dma_start(xh[:, : 2 * w], XHB[:, 2 * off : 2 * off + 2 * w])
                xs = xh[:, 0:w]
                hs = xh[:, w : 2 * w]

                if mode == "dma":
                    o = mid.tile([H, FREE], bf16, tag="o")
                    nc.vector.tensor_copy(o[:, :w], hs)
                    nc.sync.dma_start(OT[:, sl], o[:, :w])
                    continue

                pz = psum.tile([H, FREE], f32, tag="pz", bufs=1)
                pr = psum.tile([H, FREE], f32, tag="pr", bufs=1)
                ph = psum.tile([H, FREE], f32, tag="ph", bufs=2)

                for ss in mm:
                    nc.tensor.matmul(pz[:, ss], wm[Wz_i], xs[:, ss],
                                     start=True, stop=False)
                    nc.tensor.matmul(pz[:, ss], wm[Uz_i], hs[:, ss],
                                     start=False, stop=True)
                for ss in mm:
                    nc.tensor.matmul(pr[:, ss], wm[Wr_i], xs[:, ss],
                                     start=True, stop=False)
                    nc.tensor.matmul(pr[:, ss], wm[Ur_i], hs[:, ss],
                                     start=False, stop=True)
                for ss in mm:
                    nc.tensor.matmul(ph[:, ss], wm[Wh_i], xs[:, ss],
                                     start=True, stop=False)

                z = mid.tile([H, FREE], bf16, tag="z")
                nc.scalar.activation(z[:, :w], pz[:, :w], Sig, bias=bz)
                r = mid.tile([H, FREE], bf16, tag="r")
                nc.scalar.activation(r[:, :w], pr[:, :w], Sig, bias=br)

                rh = mid.tile([H, FREE], bf16, tag="rh")
                nc.vector.tensor_tensor(rh[:, :w], r[:, :w], hs, Mult)

                if carry is not None:
                    emit_tail(carry)
                carry = dict(ph=ph, rh=rh, z=z, hs=hs, sl=sl, w=w, mm=mm)
            if carry is not None:
                emit_tail(carry)

    nc.compile()
    return nc


def _get_program(n_passes=1, mode="full", cfg=None):
    def freeze(v):
        return tuple(v) if isinstance(v, (list, tuple)) else v

    key = (n_passes, mode,
           tuple(sorted((k, freeze(v)) for k, v in (cfg or CONFIG).items())))
    if key not in _CACHE:
        _CACHE[key] = _build_program(n_passes, mode, cfg)
    return _CACHE[key]


def make_in_maps(x_t, h_prev, Wz, Uz, bz, Wr, Ur, br, Wh, Uh, bh, cfg=None):
    import ml_dtypes

    cfg = dict(CONFIG, **(cfg or {}))
    bf = ml_dtypes.bfloat16

    wbb = np.empty((H, 6 * H + 3), dtype=bf)
    for i, w in enumerate((Wz, Uz, Wr, Ur, Wh, Uh)):
        wbb[:, i * H : (i + 1) * H] = np.asarray(w, dtype=np.float32).astype(bf)
    for i, b in enumerate((bz, br, bh)):
        wbb[:, 6 * H + i] = np.asarray(b, dtype=np.float32).astype(bf)

    xT = np.asarray(x_t, dtype=np.float32).T.astype(bf)
    hT = np.asarray(h_prev, dtype=np.float32).T.astype(bf)

    chunks = _chunks(cfg)
    in_maps = []
    for c in range(NCORES):
        s0 = c * BC
        xhb = np.empty((H, 2 * BC), dtype=bf)
        for off, w in chunks:
            xhb[:, 2 * off : 2 * off + w] = xT[:, s0 + off : s0 + off + w]
            xhb[:, 2 * off + w : 2 * off + 2 * w] = hT[:, s0 + off : s0 + off + w]
        in_maps.append({"XHB": xhb, "WBB": wbb})
    return in_maps


def kernel(x_t, h_prev, Wz, Uz, bz, Wr, Ur, br, Wh, Uh, bh):
    global LAST_RESULTS
    from concourse import bass_utils

    in_maps = make_in_maps(x_t, h_prev, Wz, Uz, bz, Wr, Ur, br, Wh, Uh, bh)
    nc = _get_program()
    res = bass_utils.run_bass_kernel_spmd(nc, in_maps, core_ids=list(range(NCORES)))
    LAST_RESULTS = res

    oT = np.concatenate([r["oT"] for r in res.results], axis=1)  # [H, B]
    return np.ascontiguousarray(oT.T.astype(np.float32))
